# revision 35
# baseline (speedup 1.0000x reference)
"""AdaptiveDisLoss Trainium2 kernel (8 NeuronCores, data-parallel over rows).

Math (shifted space; identical to reference after algebra):
  x' = x - x_label per row (HOST-side subtract; free, more accurate in f32).
  e' = exp(x'); s' = sum_j e'_j  (= 1/p_true)
  row_sum = 81*ln(s') - L - 5,  L = sum_j ln(max(1 - e'_j, alpha*s'))
  max(1-e', a*s') == 1 - min(e', 1 - a*s'), so ACT Ln(scale=-1, bias=1.0)
  fuses the subtract; per_true==5 folds into the -5 (Ln scale exp(-5/81)).
  contrib = (1 - 1/s')^2 * row_sum  (reference's clip(.,1e-4,1) never binds).
  Three masked sums -> host divide/clamp.

Layout: class-middle tiles [128 part, 81 class, r_t rows], host pre-permutes
and stages x' as bf16. ACT ACTIVATE is 1x/cycle/lane (measured; dtype-
independent), so the two full-width ACT passes (exp, ln) dominate. To cut
the Ln pass, K=36 class pairs are folded before Ln: TS computes em1 = e-1
(4x bf16), min vs broadcast c1m = -alpha*s' gives m1 = m-1 <= 0 (2x), and a
pair TT-mult (m1_a * m1_b = u_a*u_b, signs cancel; 2x) halves those columns.
Ln then runs on 36 pair-products (scale=+1) + 9 unpaired m1 cols
(scale=-1): 45 cols instead of 81. Segmented row-sums on PE: accumulating
identity matmuls (chunk W=6 classes) fold columns into 6 groups in PSUM;
DVE finishes with one strided reduce [128, r, 6]. PE sums raw e (s' =
sum e). RTS tapers [32,48,56,56,40,24] for ramp + short tail; tile-0 DMA is
split across two queue engines to land sooner. Ln lags exp by one tile so
ACT streams; ACT tables steered to the single exp+ln set. Epilogue runs in
thirds as L columns complete; masked sums use scalar_tensor_tensor
accum_out; 1/s' via reciprocal_approx_fast (51 ULP, plenty for the 2e-2
gate; s'>1 so no edge cases).

Measured engine busy (v2, no fold): ACT 39.0 / DVE 25.3 / PE 49 (dur
overlaps LDW). ACT = (FD+352)/1.2 ns exactly; DVE TT 2x = (FD/2+151)/0.96.
"""

import numpy as np

try:
    import concourse  # noqa: F401
except ImportError:
    import sys

    for _p in ("/opt/trn_rl_repo", "/root/.axon_site/_ro/trn_rl_repo"):
        if _p not in sys.path:
            sys.path.insert(0, _p)

import concourse.bass as bass
import concourse.bacc as bacc
import concourse.tile as tile
from concourse import mybir
from concourse.bass_utils import run_bass_kernel_spmd

N = 262144
C = 81
NUM_BASE = 60
NUM_CLASSES = 80
N_CORES = 8
NSH = N // N_CORES          # 32768 rows per core
RTS = [16, 48, 56, 56, 56, 24]   # rows/partition per tile (sum 256)
T = len(RTS)
OFFS = [0]
for _r in RTS:
    OFFS.append(OFFS[-1] + _r)
NCOL = OFFS[-1]             # 256 per-row buffer columns
FDS = [C * r for r in RTS]  # elems per partition per tile
FDMAX = max(FDS)
XOFF = [C * o for o in OFFS]
ALPHA = float(np.exp(-5.0))
K = 36                      # class pairs folded before Ln (pairs (c, c+K))
NUNP = C - 2 * K            # 9 unpaired classes at cols [2K, C)
LW = K + NUNP               # 45 Ln columns per row (45 = 9*5: clean chunks)
WARMUP_MM = 16              # dummy matmuls to lift PE out of HAM throttle
# epilogue thirds: column ranges and the tile index whose L completes them
EPI = [(slice(0, OFFS[3]), 2, 0), (slice(OFFS[3], OFFS[5]), 4, 4),
       (slice(OFFS[5], NCOL), 5, 8)]

W_NOVEL = 1.0 / 10
W_BASE = W_NOVEL / 3.0
W_NEG = 0.001

F32 = mybir.dt.float32
BF16 = mybir.dt.bfloat16
Alu = mybir.AluOpType
Act = mybir.ActivationFunctionType

_CACHE = {}

# Steer the ACT table-load placement to the single set containing exp and ln
# ("natural_log_exp_and_others") so interleaved Exp/Ln emit one
# ACT_TABLE_LOAD instead of thrashing. Set IDs are positional indexes into
# act_info.json, so the dict keeps its size and order; only the advertised
# function lists of the other entries shrink.
_STEER = {mybir.ActivationFunctionType.Exp, mybir.ActivationFunctionType.Ln,
          mybir.ActivationFunctionType.Square}


def _steered_tables(arch):
    import concourse.hw_specs as hw_specs

    tabs = hw_specs.get_activation_tables(arch)
    return {
        name: (funcs if name == "natural_log_exp_and_others" else funcs - _STEER)
        for name, funcs in tabs.items()
    }


def _build_program(ngroups):
    _orig = bacc.get_activation_tables
    bacc.get_activation_tables = _steered_tables
    try:
        return _build_program_inner(ngroups)
    finally:
        bacc.get_activation_tables = _orig


def _build_program_inner(ngroups):
    nc = bacc.Bacc()
    x_in = nc.declare_dram_parameter("x", [128, XOFF[-1]], BF16, isOutput=False)
    mk_in = nc.declare_dram_parameter(
        "mk", [128, ngroups * NCOL], F32, isOutput=False
    )
    idm_in = nc.declare_dram_parameter("idm", [128, 128], BF16, isOutput=False)
    out_d = nc.declare_dram_parameter("out", [128, 12], F32, isOutput=True)

    with tile.TileContext(nc) as tc:
        with (
            tc.tile_pool(name="persist", bufs=1) as persist,
            tc.tile_pool(name="px", bufs=3) as px,
            tc.tile_pool(name="pe", bufs=2) as pe,
            tc.tile_pool(name="pm", bufs=2) as pm,
            tc.tile_pool(name="pmm", bufs=2) as pmm,
            tc.tile_pool(name="pg", bufs=2) as pg,
            tc.tile_pool(name="pl", bufs=2) as pl,
            tc.tile_pool(name="pep", bufs=1) as pep,
            tc.psum_pool(name="psS", bufs=2) as psS,
            tc.psum_pool(name="psL", bufs=2) as psL,
            tc.psum_pool(name="psW", bufs=1) as psW,
        ):
            mk_sb = persist.tile([128, ngroups * NCOL], F32)
            idb = persist.tile([128, 128], BF16)   # holds -I (negated identity)
            snegb = persist.tile([128, NCOL], BF16)  # -s' per row
            L_buf = persist.tile([128, NCOL], F32)   # -L per row (via -I sums)
            osb = persist.tile([128, 12], F32)
            nc.vector.memset(osb, 0.0)

            # PE warm-up: HAM clock-gates an idle PE to 1.2 GHz and only
            # releases after ~3.4us of sustained activity. Burn dummy matmuls
            # on scratch data while the first x tile is still in flight so
            # the real matmuls run at 2.4 GHz from the start.
            if WARMUP_MM:
                wscr = persist.tile([128, 128], BF16)
                mscr = persist.tile([128, 504], BF16)
                wsink = persist.tile([128, 1], F32)
                nc.vector.memset(wscr, 0.0)
                nc.vector.memset(mscr, 0.0)
                pswt = psW.tile([128, 504], F32, tag="warm")
                for i in range(WARMUP_MM):
                    nc.tensor.matmul(
                        pswt, wscr, mscr,
                        start=(i == 0), stop=(i == WARMUP_MM - 1),
                    )
                nc.vector.tensor_reduce(
                    wsink, pswt[:, 0:4], axis=mybir.AxisListType.X, op=Alu.add
                )

            def bcast(buf, t):
                r = RTS[t]
                return buf[:, OFFS[t] : OFFS[t + 1]].rearrange(
                    "p (o r) -> p o r", o=1
                ).to_broadcast([128, C, r])

            def seg_sum(pool, src_bf, t, dst, ncols, w):
                """PE: fold `ncols` class-columns into `w` groups in PSUM
                (ncols/w accumulating identity matmuls, FD=w*r each; equal
                out-APs per accum group is a NEFF-load requirement);
                DVE: one strided reduce [128, r, w]."""
                r = RTS[t]
                nmm = ncols // w
                assert ncols == w * nmm
                psf = pool.tile([128, 9 * max(RTS)], F32, tag="ps")
                ps = psf[:, 0 : w * r]
                for k in range(nmm):
                    nc.tensor.matmul(
                        ps, idb, src_bf[:, w * r * k : w * r * (k + 1)],
                        start=(k == 0), stop=(k == nmm - 1),
                    )
                psv = ps.rearrange("p (cc r) -> p r cc", cc=w)
                with nc.allow_low_precision(reason="bf16 s' is ample for 2e-2"):
                    nc.vector.tensor_reduce(
                        dst, psv, axis=mybir.AxisListType.X, op=Alu.add
                    )

            def emit_ln(t, gs, m1s):
                """Ln on K pair-products (g = u_a*u_b/alpha^2: scale e^-10
                re-centers) + NUNP unpaired m1p cols (-m1p = u/alpha: scale
                -e^-5); then (negated) L row-sums via the -I seg-sum."""
                r = RTS[t]
                ltf = pl.tile([128, LW * max(RTS)], BF16, tag="lt")
                lt = ltf[:, 0 : LW * r]
                nc.scalar.activation(
                    lt[:, 0 : K * r], gs[t], Act.Ln, scale=float(np.exp(-10.0))
                )
                nc.scalar.activation(
                    lt[:, K * r : LW * r], m1s[t], Act.Ln,
                    scale=-float(np.exp(-5.0)),
                )
                seg_sum(psL, lt, t, L_buf[:, OFFS[t] : OFFS[t + 1]], LW, 5)

            def emit_epilogue(h, cs, og):
                """Epilogue over column slice cs; masked sums into osb[:, og:og+3].
                contrib = (1-1/s')^2 * (81*ln(s') - L - 5)."""
                nco = cs.stop - cs.start
                logs = pep.tile([128, nco], F32, tag=f"logs{h}")
                nc.scalar.activation(
                    logs, snegb[:, cs], Act.Ln, scale=-float(np.exp(-5.0 / C))
                )
                scp = pep.tile([128, nco], F32, tag=f"scp{h}")
                nc.vector.tensor_copy(scp, snegb[:, cs])
                rinv = pep.tile([128, nco], F32, tag=f"rinv{h}")
                nc.vector.reciprocal_approx_fast(out=rinv, in_=scp)  # -1/s'
                ompc = pep.tile([128, nco], F32, tag=f"ompc{h}")
                nc.vector.tensor_scalar(
                    out=ompc, in0=rinv, scalar1=1.0, scalar2=1.0,
                    op0=Alu.mult, op1=Alu.add,
                )
                w = pep.tile([128, nco], F32, tag=f"w{h}")
                nc.vector.tensor_tensor(out=w, in0=ompc, in1=ompc, op=Alu.mult)
                rs2 = pep.tile([128, nco], F32, tag=f"rs2{h}")
                nc.vector.scalar_tensor_tensor(
                    out=rs2, in0=logs, scalar=float(C), in1=L_buf[:, cs],
                    op0=Alu.mult, op1=Alu.add,
                )
                contrib = pep.tile([128, nco], F32, tag=f"con{h}")
                if ngroups == 2:
                    # contrib + its full-row sum in one op; the neg group is
                    # recovered on the host as tot - base - novel (the three
                    # groups partition all rows when coverage holds).
                    nc.vector.tensor_tensor_reduce(
                        out=contrib, in0=w, in1=rs2, scale=1.0, scalar=0.0,
                        op0=Alu.mult, op1=Alu.add,
                        accum_out=osb[:, og + 2 : og + 3],
                    )
                else:
                    nc.vector.tensor_tensor(
                        out=contrib, in0=w, in1=rs2, op=Alu.mult
                    )
                for g in range(ngroups):
                    scr = pep.tile([128, nco], F32, tag=f"scr{h}{g}")
                    nc.vector.scalar_tensor_tensor(
                        out=scr, in0=contrib, scalar=1.0,
                        in1=mk_sb[:, g * NCOL + cs.start : g * NCOL + cs.stop],
                        op0=Alu.mult, op1=Alu.mult,
                        accum_out=osb[:, og + g : og + g + 1],
                    )

            # pipeline: Ln lags exp by one tile so ACT streams continuously
            gs = {}
            m1s = {}
            for t in range(T):
                fd = FDS[t]
                r = RTS[t]
                rcols = slice(OFFS[t], OFFS[t + 1])
                xbf = px.tile([128, FDMAX], BF16, tag="xb")
                xb = xbf[:, 0:fd]
                # alternate queues so consecutive tiles transfer in parallel
                qeng = nc.sync if t % 2 == 0 else nc.gpsimd
                qeng.dma_start(out=xb, in_=x_in[:, XOFF[t] : XOFF[t + 1]])
                if t == 0:
                    nc.sync.dma_start(out=idb, in_=idm_in[:])
                if t == 2:
                    # masks are first needed by the t==2 epilogue third
                    nc.sync.dma_start(out=mk_sb, in_=mk_in[:])
                etf = pe.tile([128, FDMAX], BF16, tag="et")
                et = etf[:, 0:fd]
                nc.scalar.activation(et, xb, Act.Exp)
                # em1s = (e-1)/alpha (4x TS) while PE sums raw e with -I;
                # then min(em1s, -s') = (min(e, 1-alpha*s') - 1)/alpha, so the
                # broadcast operand is the reduce output directly (no scale op)
                emf = pm.tile([128, FDMAX], BF16, tag="em")
                em1 = emf[:, 0:fd]
                nc.vector.tensor_scalar(
                    out=em1, in0=et, scalar1=1.0 / ALPHA, scalar2=1.0 / ALPHA,
                    op0=Alu.mult, op1=Alu.subtract,
                )
                seg_sum(psS, et, t, snegb[:, rcols], C, 9)
                mtf = pmm.tile([128, FDMAX], BF16, tag="m1")
                mt = mtf[:, 0:fd]
                nc.vector.tensor_tensor(
                    out=mt.rearrange("p (c r) -> p c r", r=r),
                    in0=em1.rearrange("p (c r) -> p c r", r=r),
                    in1=bcast(snegb, t),
                    op=Alu.min,
                )
                # pair fold: m1p_a*m1p_b = u_a*u_b/alpha^2 > 0, pairs (c, c+K)
                gf = pg.tile([128, K * max(RTS)], BF16, tag="g")
                g = gf[:, 0 : K * r]
                nc.vector.tensor_tensor(
                    out=g, in0=mt[:, 0 : K * r],
                    in1=mt[:, K * r : 2 * K * r], op=Alu.mult,
                )
                gs[t] = g
                m1s[t] = mt[:, 2 * K * r : C * r]
                if t >= 1:
                    emit_ln(t - 1, gs, m1s)
                for cs, tw, og in EPI:
                    if tw == t - 1:
                        emit_epilogue(og // 4, cs, og)
                if WARMUP_MM and t < T - 1:
                    # keep HAM from re-throttling across the inter-tile PE gap
                    for i in range(6):
                        nc.tensor.matmul(
                            pswt, wscr, mscr, start=(i == 0), stop=(i == 5)
                        )
            emit_ln(T - 1, gs, m1s)
            emit_epilogue(2, EPI[2][0], 8)

            nc.sync.dma_start(out=out_d[:], in_=osb)

    nc.finalize()
    return nc


def _get_program(ngroups):
    key = f"nc{ngroups}"
    if key not in _CACHE:
        _CACHE[key] = _build_program(ngroups)
    return _CACHE[key]


def _row_layout(a):
    """[NSH] -> [128, NCOL]; tile t holds rows [128*OFFS[t], 128*OFFS[t+1])
    as [128, r_t] (partition-major), at cols [OFFS[t], OFFS[t+1])."""
    pieces = []
    for t in range(T):
        seg = a[128 * OFFS[t] : 128 * OFFS[t + 1]].reshape(128, RTS[t])
        pieces.append(seg)
    return np.ascontiguousarray(np.concatenate(pieces, axis=1))


def prepare_inputs(cls_score, labels, label_weights):
    import ml_dtypes

    x = np.ascontiguousarray(np.asarray(cls_score, dtype=np.float32))
    lab = np.asarray(labels).astype(np.int64)
    lw = np.asarray(label_weights, dtype=np.float32)

    valid = lw > 0
    counts = np.bincount(lab[valid], minlength=C)
    enough = counts[lab] >= 2
    base_sel = valid & (lab < NUM_BASE) & enough
    novel_sel = valid & (lab >= NUM_BASE) & (lab < NUM_CLASSES) & enough
    neg_sel = valid & (lab == NUM_CLASSES)

    xl = np.take_along_axis(x, lab[:, None], axis=1)[:, 0]
    xs_full = x - xl[:, None]  # host-side shift: x' = x - x_label
    # NOTE: a 2-mask variant (neg = total - base - novel via
    # tensor_tensor_reduce accum) fails at NEFF runtime - keep 3 masks.
    ngroups = 3
    masks = np.stack(
        [base_sel.astype(np.float32), novel_sel.astype(np.float32),
         neg_sel.astype(np.float32)]
    )

    idm = (-np.eye(128, dtype=np.float32)).astype(ml_dtypes.bfloat16)
    in_maps = []
    for i in range(N_CORES):
        sl = slice(i * NSH, (i + 1) * NSH)
        xs = xs_full[sl]
        xpieces = []
        for t in range(T):
            seg = xs[128 * OFFS[t] : 128 * OFFS[t + 1]]  # [128*r_t, C]
            xpieces.append(
                seg.reshape(128, RTS[t], C).transpose(0, 2, 1).reshape(128, FDS[t])
            )
        xc = np.concatenate(xpieces, axis=1)  # [128, sum FD]
        mk = np.concatenate(
            [_row_layout(masks[g, sl]) for g in range(ngroups)], axis=1
        )
        in_maps.append(
            {
                "x": np.ascontiguousarray(xc).astype(ml_dtypes.bfloat16),
                "mk": np.ascontiguousarray(mk),
                "idm": idm,
            }
        )
    ns = (int(base_sel.sum()), int(novel_sel.sum()), int(neg_sel.sum()))
    return in_maps, ns, ngroups


def finalize(results, ns, ngroups):
    sums = np.zeros(3, dtype=np.float64)
    for r in results:
        o = np.asarray(r["out"], dtype=np.float64)
        sums += o[:, 0:3].sum(axis=0) + o[:, 4:7].sum(axis=0) + o[:, 8:11].sum(axis=0)
    if ngroups == 2:
        # cols: 0=base, 1=novel, 2=total -> neg = tot - base - novel
        sums = np.array([sums[0], sums[1], sums[2] - sums[0] - sums[1]])
    losses = []
    for g, wg in enumerate((W_BASE, W_NOVEL, W_NEG)):
        n = ns[g]
        if n > 0:
            mean = sums[g] / (max(n, 1) * (C - 1))
        else:
            mean = 0.0
        losses.append(np.float32(min(mean * wg, 1.0)))
    return tuple(losses)


def kernel(cls_score, labels, label_weights, _trace=False, _tmpdir=None):
    in_maps, ns, ngroups = prepare_inputs(cls_score, labels, label_weights)
    nc = _get_program(ngroups)
    res = run_bass_kernel_spmd(
        nc, in_maps, core_ids=list(range(N_CORES)), trace=_trace, tmpdir=_tmpdir
    )
    out = finalize(res.results, ns, ngroups)
    if _trace:
        return out, res
    return out


# revision 40
# speedup vs baseline: 1.1787x; 1.1787x over previous
"""AdaptiveDisLoss Trainium2 kernel (8 NeuronCores, data-parallel over rows).

Math (shifted space; identical to reference after algebra):
  x' = x - x_label per row (HOST-side subtract; free, more accurate in f32).
  e' = exp(x'); s' = sum_j e'_j  (= 1/p_true)
  row_sum = 81*ln(s') - L - 5,  L = sum_j ln(max(1 - e'_j, alpha*s'))
  max(1-e', a*s') == 1 - min(e', 1 - a*s'), so ACT Ln(scale=-1, bias=1.0)
  fuses the subtract; per_true==5 folds into the -5 (Ln scale exp(-5/81)).
  contrib = (1 - 1/s')^2 * row_sum  (reference's clip(.,1e-4,1) never binds).
  Three masked sums -> host divide/clamp.

Layout: class-middle tiles [128 part, 81 class, r_t rows], host pre-permutes
and stages x' as bf16. ACT ACTIVATE is 1x/cycle/lane = (FD+352)/1.2 ns
(measured; dtype-independent), so the two full-width ACT passes (exp, ln)
dominate. To cut the Ln pass, K=36 class pairs fold before Ln, all in
"scaled (e-1)" space: TS computes em1s = (e-1)/alpha (4x bf16); PE sums raw
e with a NEGATED identity so the strided reduce emits sneg = -s' in bf16,
which is directly the min comparand: min(em1s, sneg) = (m-1)/alpha =: m1p
(broadcast TT, 2x). A pair TT-mult (m1p_a*m1p_b = u_a*u_b/alpha^2 > 0; 2x)
folds pairs (c, c+36); Ln runs on 36 products (scale=e^-10) + 9 unpaired
cols (scale=-e^-5): 45 cols instead of 81. L row-sums reuse the -I seg-sum
(L_buf = -L, so rs2 = 81*logs + L_buf). PE seg-sums: chunk W=9 (S) / W=5
(L) accumulating matmuls into PSUM + one DVE strided reduce [128, r, W];
equal out-APs per accum group (NEFF-load requirement) force W | ncols.
16 warm-up + 6-per-gap keep-warm dummy matmuls hold the PE HAM clock at
2.4 GHz (idle >3.4us re-throttles to 1.2). RTS tapers [16,48,56,56,56,24]
for DMA ramp + short exit tail; px bufs=3 lets DMA prefetch 2 tiles ahead.
Ln lags exp by one tile so ACT streams; ACT tables steered to the single
exp+ln set. Epilogue runs in thirds as L columns complete; masked sums use
scalar_tensor_tensor accum_out; 1/s' via reciprocal_approx_fast (51 ULP,
ample for the 2e-2 gate; |s'|>1 so no edge cases).

Measured (cool chip): 54.6us total = ~10.8 ramp (7.0 NEFF preamble + DMA
spin-up) + ~40 compute window (ACT busy 33, DVE 37, PE overlapped) + ~3.9
semaphore teardown. Chip thermal throttling adds up to +20% run-to-run.
Tried and rejected: W=3 chunks (DVE reduce -3us but +350 instructions ->
+2.3us NEFF instruction-prefetch preamble); 2-mask total-trick via
tensor_tensor_reduce (NEFF runtime failure); fp8/custom ACT tables
(runtime tables are pre-baked); gpsimd offload (no generic elementwise
ops in this bass); DVE TT is capped at 2x (no 4x uop), STT/reduce at 1x.
"""

import numpy as np

try:
    import concourse  # noqa: F401
except ImportError:
    import sys

    for _p in ("/opt/trn_rl_repo", "/root/.axon_site/_ro/trn_rl_repo"):
        if _p not in sys.path:
            sys.path.insert(0, _p)

import concourse.bass as bass
import concourse.bacc as bacc
import concourse.tile as tile
from concourse import mybir
from concourse.bass_utils import run_bass_kernel_spmd

N = 262144
C = 81
NUM_BASE = 60
NUM_CLASSES = 80
N_CORES = 8
NSH = N // N_CORES          # 32768 rows per core
RTS = [16, 48, 56, 56, 56, 24]   # rows/partition per tile (sum 256)
T = len(RTS)
OFFS = [0]
for _r in RTS:
    OFFS.append(OFFS[-1] + _r)
NCOL = OFFS[-1]             # 256 per-row buffer columns
FDS = [C * r for r in RTS]  # elems per partition per tile
FDMAX = max(FDS)
XOFF = [C * o for o in OFFS]
ALPHA = float(np.exp(-5.0))
K = 36                      # class pairs folded before Ln (pairs (c, c+K))
NUNP = C - 2 * K            # 9 unpaired classes at cols [2K, C)
LW = K + NUNP               # 45 Ln columns per row (45 = 9*5: clean chunks)
WARMUP_MM = 16              # dummy matmuls to lift PE out of HAM throttle
# epilogue thirds: column ranges and the tile index whose L completes them
EPI = [(slice(0, OFFS[3]), 2, 0), (slice(OFFS[3], OFFS[5]), 4, 4),
       (slice(OFFS[5], NCOL), 5, 8)]

W_NOVEL = 1.0 / 10
W_BASE = W_NOVEL / 3.0
W_NEG = 0.001

F32 = mybir.dt.float32
BF16 = mybir.dt.bfloat16
Alu = mybir.AluOpType
Act = mybir.ActivationFunctionType

_CACHE = {}

# Steer the ACT table-load placement to the single set containing exp and ln
# ("natural_log_exp_and_others") so interleaved Exp/Ln emit one
# ACT_TABLE_LOAD instead of thrashing. Set IDs are positional indexes into
# act_info.json, so the dict keeps its size and order; only the advertised
# function lists of the other entries shrink.
_STEER = {mybir.ActivationFunctionType.Exp, mybir.ActivationFunctionType.Ln,
          mybir.ActivationFunctionType.Square}


def _steered_tables(arch):
    import concourse.hw_specs as hw_specs

    tabs = hw_specs.get_activation_tables(arch)
    return {
        name: (funcs if name == "natural_log_exp_and_others" else funcs - _STEER)
        for name, funcs in tabs.items()
    }


def _build_program(ngroups):
    _orig = bacc.get_activation_tables
    bacc.get_activation_tables = _steered_tables
    try:
        return _build_program_inner(ngroups)
    finally:
        bacc.get_activation_tables = _orig


def _build_program_inner(ngroups):
    nc = bacc.Bacc()
    x_in = nc.declare_dram_parameter("x", [128, XOFF[-1]], BF16, isOutput=False)
    mk_in = nc.declare_dram_parameter(
        "mk", [128, ngroups * NCOL], F32, isOutput=False
    )
    idm_in = nc.declare_dram_parameter("idm", [128, 128], BF16, isOutput=False)
    out_d = nc.declare_dram_parameter("out", [128, 12], F32, isOutput=True)

    with tile.TileContext(nc) as tc:
        with (
            tc.tile_pool(name="persist", bufs=1) as persist,
            tc.tile_pool(name="px", bufs=3) as px,
            tc.tile_pool(name="pe", bufs=2) as pe,
            tc.tile_pool(name="pm", bufs=2) as pm,
            tc.tile_pool(name="pmm", bufs=2) as pmm,
            tc.tile_pool(name="pg", bufs=2) as pg,
            tc.tile_pool(name="pl", bufs=2) as pl,
            tc.tile_pool(name="pep", bufs=1) as pep,
            tc.psum_pool(name="psS", bufs=2) as psS,
            tc.psum_pool(name="psL", bufs=2) as psL,
            tc.psum_pool(name="psW", bufs=1) as psW,
        ):
            mk_sb = persist.tile([128, ngroups * NCOL], F32)
            idb = persist.tile([128, 128], BF16)   # holds -I (negated identity)
            snegb = persist.tile([128, NCOL], BF16)  # -s' per row
            L_buf = persist.tile([128, NCOL], F32)   # -L per row (via -I sums)
            osb = persist.tile([128, 12], F32)
            nc.vector.memset(osb, 0.0)

            # PE warm-up: HAM clock-gates an idle PE to 1.2 GHz and only
            # releases after ~3.4us of sustained activity. Burn dummy matmuls
            # on scratch data while the first x tile is still in flight so
            # the real matmuls run at 2.4 GHz from the start.
            if WARMUP_MM:
                wscr = persist.tile([128, 128], BF16)
                mscr = persist.tile([128, 504], BF16)
                wsink = persist.tile([128, 1], F32)
                nc.vector.memset(wscr, 0.0)
                nc.vector.memset(mscr, 0.0)
                pswt = psW.tile([128, 504], F32, tag="warm")
                for i in range(WARMUP_MM):
                    nc.tensor.matmul(
                        pswt, wscr, mscr,
                        start=(i == 0), stop=(i == WARMUP_MM - 1),
                    )
                nc.vector.tensor_reduce(
                    wsink, pswt[:, 0:4], axis=mybir.AxisListType.X, op=Alu.add
                )

            def bcast(buf, t):
                r = RTS[t]
                return buf[:, OFFS[t] : OFFS[t + 1]].rearrange(
                    "p (o r) -> p o r", o=1
                ).to_broadcast([128, C, r])

            def seg_sum(pool, src_bf, t, dst, ncols, w):
                """PE: fold `ncols` class-columns into `w` groups in PSUM
                (ncols/w accumulating identity matmuls, FD=w*r each; equal
                out-APs per accum group is a NEFF-load requirement);
                DVE: one strided reduce [128, r, w]."""
                r = RTS[t]
                nmm = ncols // w
                assert ncols == w * nmm
                psf = pool.tile([128, 9 * max(RTS)], F32, tag="ps")
                ps = psf[:, 0 : w * r]
                for k in range(nmm):
                    nc.tensor.matmul(
                        ps, idb, src_bf[:, w * r * k : w * r * (k + 1)],
                        start=(k == 0), stop=(k == nmm - 1),
                    )
                psv = ps.rearrange("p (cc r) -> p r cc", cc=w)
                with nc.allow_low_precision(reason="bf16 s' is ample for 2e-2"):
                    nc.vector.tensor_reduce(
                        dst, psv, axis=mybir.AxisListType.X, op=Alu.add
                    )

            def emit_ln(t, gs, m1s):
                """Ln on K pair-products (g = u_a*u_b/alpha^2: scale e^-10
                re-centers) + NUNP unpaired m1p cols (-m1p = u/alpha: scale
                -e^-5); then (negated) L row-sums via the -I seg-sum."""
                r = RTS[t]
                ltf = pl.tile([128, LW * max(RTS)], BF16, tag="lt")
                lt = ltf[:, 0 : LW * r]
                nc.scalar.activation(
                    lt[:, 0 : K * r], gs[t], Act.Ln, scale=float(np.exp(-10.0))
                )
                nc.scalar.activation(
                    lt[:, K * r : LW * r], m1s[t], Act.Ln,
                    scale=-float(np.exp(-5.0)),
                )
                seg_sum(psL, lt, t, L_buf[:, OFFS[t] : OFFS[t + 1]], LW, 5)

            def emit_epilogue(h, cs, og):
                """Epilogue over column slice cs; masked sums into osb[:, og:og+3].
                contrib = (1-1/s')^2 * (81*ln(s') - L - 5)."""
                nco = cs.stop - cs.start
                logs = pep.tile([128, nco], F32, tag=f"logs{h}")
                nc.scalar.activation(
                    logs, snegb[:, cs], Act.Ln, scale=-float(np.exp(-5.0 / C))
                )
                scp = pep.tile([128, nco], F32, tag=f"scp{h}")
                nc.vector.tensor_copy(scp, snegb[:, cs])
                rinv = pep.tile([128, nco], F32, tag=f"rinv{h}")
                nc.vector.reciprocal_approx_fast(out=rinv, in_=scp)  # -1/s'
                ompc = pep.tile([128, nco], F32, tag=f"ompc{h}")
                nc.vector.tensor_scalar(
                    out=ompc, in0=rinv, scalar1=1.0, scalar2=1.0,
                    op0=Alu.mult, op1=Alu.add,
                )
                w = pep.tile([128, nco], F32, tag=f"w{h}")
                nc.vector.tensor_tensor(out=w, in0=ompc, in1=ompc, op=Alu.mult)
                rs2 = pep.tile([128, nco], F32, tag=f"rs2{h}")
                nc.vector.scalar_tensor_tensor(
                    out=rs2, in0=logs, scalar=float(C), in1=L_buf[:, cs],
                    op0=Alu.mult, op1=Alu.add,
                )
                contrib = pep.tile([128, nco], F32, tag=f"con{h}")
                nc.vector.tensor_tensor(
                    out=contrib, in0=w, in1=rs2, op=Alu.mult
                )
                for g in range(ngroups):
                    scr = pep.tile([128, nco], F32, tag=f"scr{h}{g}")
                    nc.vector.scalar_tensor_tensor(
                        out=scr, in0=contrib, scalar=1.0,
                        in1=mk_sb[:, g * NCOL + cs.start : g * NCOL + cs.stop],
                        op0=Alu.mult, op1=Alu.mult,
                        accum_out=osb[:, og + g : og + g + 1],
                    )

            # pipeline: Ln lags exp by one tile so ACT streams continuously
            gs = {}
            m1s = {}
            for t in range(T):
                fd = FDS[t]
                r = RTS[t]
                rcols = slice(OFFS[t], OFFS[t + 1])
                xbf = px.tile([128, FDMAX], BF16, tag="xb")
                xb = xbf[:, 0:fd]
                nc.sync.dma_start(out=xb, in_=x_in[:, XOFF[t] : XOFF[t + 1]])
                if t == 0:
                    nc.gpsimd.dma_start(out=idb, in_=idm_in[:])
                if t == 2:
                    # masks are first needed by the t==2 epilogue third
                    nc.gpsimd.dma_start(out=mk_sb, in_=mk_in[:])
                etf = pe.tile([128, FDMAX], BF16, tag="et")
                et = etf[:, 0:fd]
                nc.scalar.activation(et, xb, Act.Exp)
                # em1s = (e-1)/alpha (4x TS) while PE sums raw e with -I;
                # then min(em1s, -s') = (min(e, 1-alpha*s') - 1)/alpha, so the
                # broadcast operand is the reduce output directly (no scale op)
                emf = pm.tile([128, FDMAX], BF16, tag="em")
                em1 = emf[:, 0:fd]
                nc.vector.tensor_scalar(
                    out=em1, in0=et, scalar1=1.0 / ALPHA, scalar2=1.0 / ALPHA,
                    op0=Alu.mult, op1=Alu.subtract,
                )
                seg_sum(psS, et, t, snegb[:, rcols], C, 9)
                mtf = pmm.tile([128, FDMAX], BF16, tag="m1")
                mt = mtf[:, 0:fd]
                nc.vector.tensor_tensor(
                    out=mt.rearrange("p (c r) -> p c r", r=r),
                    in0=em1.rearrange("p (c r) -> p c r", r=r),
                    in1=bcast(snegb, t),
                    op=Alu.min,
                )
                # pair fold: m1p_a*m1p_b = u_a*u_b/alpha^2 > 0, pairs (c, c+K)
                gf = pg.tile([128, K * max(RTS)], BF16, tag="g")
                g = gf[:, 0 : K * r]
                nc.vector.tensor_tensor(
                    out=g, in0=mt[:, 0 : K * r],
                    in1=mt[:, K * r : 2 * K * r], op=Alu.mult,
                )
                gs[t] = g
                m1s[t] = mt[:, 2 * K * r : C * r]
                if t >= 1:
                    emit_ln(t - 1, gs, m1s)
                for cs, tw, og in EPI:
                    if tw == t - 1:
                        emit_epilogue(og // 4, cs, og)
                if WARMUP_MM and t < T - 1:
                    # keep HAM from re-throttling across the inter-tile PE gap
                    for i in range(6):
                        nc.tensor.matmul(
                            pswt, wscr, mscr, start=(i == 0), stop=(i == 5)
                        )
            emit_ln(T - 1, gs, m1s)
            emit_epilogue(2, EPI[2][0], 8)

            nc.sync.dma_start(out=out_d[:], in_=osb)

    nc.finalize()
    return nc


def _get_program(ngroups):
    key = f"nc{ngroups}"
    if key not in _CACHE:
        _CACHE[key] = _build_program(ngroups)
    return _CACHE[key]


def _row_layout(a):
    """[NSH] -> [128, NCOL]; tile t holds rows [128*OFFS[t], 128*OFFS[t+1])
    as [128, r_t] (partition-major), at cols [OFFS[t], OFFS[t+1])."""
    pieces = []
    for t in range(T):
        seg = a[128 * OFFS[t] : 128 * OFFS[t + 1]].reshape(128, RTS[t])
        pieces.append(seg)
    return np.ascontiguousarray(np.concatenate(pieces, axis=1))


def prepare_inputs(cls_score, labels, label_weights):
    import ml_dtypes

    x = np.ascontiguousarray(np.asarray(cls_score, dtype=np.float32))
    lab = np.asarray(labels).astype(np.int64)
    lw = np.asarray(label_weights, dtype=np.float32)

    valid = lw > 0
    counts = np.bincount(lab[valid], minlength=C)
    enough = counts[lab] >= 2
    base_sel = valid & (lab < NUM_BASE) & enough
    novel_sel = valid & (lab >= NUM_BASE) & (lab < NUM_CLASSES) & enough
    neg_sel = valid & (lab == NUM_CLASSES)

    xl = np.take_along_axis(x, lab[:, None], axis=1)[:, 0]
    xs_full = x - xl[:, None]  # host-side shift: x' = x - x_label
    # NOTE: a 2-mask variant (neg = total - base - novel via
    # tensor_tensor_reduce accum) fails at NEFF runtime - keep 3 masks.
    ngroups = 3
    masks = np.stack(
        [base_sel.astype(np.float32), novel_sel.astype(np.float32),
         neg_sel.astype(np.float32)]
    )

    idm = (-np.eye(128, dtype=np.float32)).astype(ml_dtypes.bfloat16)
    in_maps = []
    for i in range(N_CORES):
        sl = slice(i * NSH, (i + 1) * NSH)
        xs = xs_full[sl]
        xpieces = []
        for t in range(T):
            seg = xs[128 * OFFS[t] : 128 * OFFS[t + 1]]  # [128*r_t, C]
            xpieces.append(
                seg.reshape(128, RTS[t], C).transpose(0, 2, 1).reshape(128, FDS[t])
            )
        xc = np.concatenate(xpieces, axis=1)  # [128, sum FD]
        mk = np.concatenate(
            [_row_layout(masks[g, sl]) for g in range(ngroups)], axis=1
        )
        in_maps.append(
            {
                "x": np.ascontiguousarray(xc).astype(ml_dtypes.bfloat16),
                "mk": np.ascontiguousarray(mk),
                "idm": idm,
            }
        )
    ns = (int(base_sel.sum()), int(novel_sel.sum()), int(neg_sel.sum()))
    return in_maps, ns, ngroups


def finalize(results, ns, ngroups):
    sums = np.zeros(3, dtype=np.float64)
    for r in results:
        o = np.asarray(r["out"], dtype=np.float64)
        sums += o[:, 0:3].sum(axis=0) + o[:, 4:7].sum(axis=0) + o[:, 8:11].sum(axis=0)
    if ngroups == 2:
        # cols: 0=base, 1=novel, 2=total -> neg = tot - base - novel
        sums = np.array([sums[0], sums[1], sums[2] - sums[0] - sums[1]])
    losses = []
    for g, wg in enumerate((W_BASE, W_NOVEL, W_NEG)):
        n = ns[g]
        if n > 0:
            mean = sums[g] / (max(n, 1) * (C - 1))
        else:
            mean = 0.0
        losses.append(np.float32(min(mean * wg, 1.0)))
    return tuple(losses)


def kernel(cls_score, labels, label_weights, _trace=False, _tmpdir=None):
    in_maps, ns, ngroups = prepare_inputs(cls_score, labels, label_weights)
    nc = _get_program(ngroups)
    res = run_bass_kernel_spmd(
        nc, in_maps, core_ids=list(range(N_CORES)), trace=_trace, tmpdir=_tmpdir
    )
    out = finalize(res.results, ns, ngroups)
    if _trace:
        return out, res
    return out
